# revision 56
# baseline (speedup 1.0000x reference)
"""Causal self-attention (dense transformer block) on 8 Trainium2 NeuronCores.

Sharding: tensor-parallel over heads x data-parallel over batch.
  - 8 cores = 2 batch groups x 4 cores; each core owns 1 batch element and
    4 of the 16 heads (head_dim 64 -> 256 local channels).
  - Host pre-transposes x and the weight slices so the device never has to
    transpose activations (PE contracts along partitions).
  - Each core computes qkv projection for its heads, causal attention in
    "S^T" layout (scores[k, q], k on partitions), and its partial c_proj.
  - Host sums the 4 partials per batch and adds the bias terms.

Math notes:
  - k-bias and v-bias never enter the kernel: the k-bias contribution to the
    scores is constant along the softmax axis (cancels exactly), and the
    v-bias passes through softmax (rows sum to 1) and c_proj into a constant
    output offset w_proj @ b_v, added on host.
  - Softmax skips the max-subtraction pass: scores/8 have |.| <~ 3 for this
    distribution, exp cannot overflow, and the result is mathematically
    identical.
  - attV is computed with V augmented by a ones column, so the softmax
    denominators fall out of the same matmul (row 64 of the PSUM tile).
  - All matmuls run in bf16 (PSUM accumulation stays fp32). fp32r draws
    enough PE power that the HAM throttles the clock to 4/8 duty for half
    the kernel; bf16 needs half the weight-load and DMA bytes and throttles
    far less. Softmax denominators are accumulated and inverted in fp32.

Scheduling notes:
  - Input DMA is t-chunk-major across three queues (v-weights first on the
    gpsimd queue, x 512-column chunks alternating sync/scalar queues) so the
    first V matmul starts after ~1.5 MB instead of ~5 MB of traffic.
  - Work is emitted per head-pair (V, Q^T/K^T, then attention) so the
    second pair's projection matmuls fill the PE gaps while the first
    pair's softmax runs on ScalarE.
  - Softmax denominators are staged on PARTITION 0 (one [1,512] slot per
    head x q-chunk) so gpsimd's partition_broadcast reads them directly --
    no bounce DMA. Each chunk is inverted with reciprocal_approx_fast
    (~5x cheaper than nc.vector.reciprocal) and normalized in place right
    when its attV completes, so the chain never convoys the DVE queue at
    head boundaries.
  - c_proj for the first T/2 rows is emitted between the last head's two
    halves: its matmuls fill the PE gaps left by half-1's softmax waits and
    only half of c_proj remains in the tail.
  - attV PSUM tiles are released right after two cheap copies; the
    normalize-multiply happens later, in place, in SBUF.
"""

import numpy as np
from contextlib import ExitStack

from ml_dtypes import bfloat16

import concourse.bass as bass
import concourse.tile as tile
from concourse import bacc, library_config, mybir
from concourse.bass_utils import run_bass_kernel_spmd

FP32 = mybir.dt.float32
BF16 = mybir.dt.bfloat16
AF = mybir.ActivationFunctionType

B, T_FULL, C = 2, 2048, 1024
H, D = 16, 64
NCORES = 8
CPG = 4          # cores per batch group
HPC = H // CPG   # heads per core = 4
HL = HPC * D     # local channels = 256
NQO = HL // 128  # head pairs per core = 2
CT = C // 128    # contraction tiles = 8


def _nsplit(w):
    """Split width into matmul N-chunks at 512-aligned offsets (a matmul
    output may not cross a PSUM bank line)."""
    chunks = [512] * (w // 512)
    if w % 512:
        chunks.append(w % 512)
    return chunks


def build_bass(T=T_FULL):
    """Emit the SPMD Bass/Tile program for one core (same program, per-core
    data). T must be a multiple of 1024 (two halves per q-range, 512-chunks)."""
    assert T == 2048  # DMA schedule below is laid out for 4 x-chunks
    TT = T // 128          # t-tiles
    HALF = T // 2
    NCH = T // 512         # 512-chunks per head

    nc = bacc.Bacc("TRN2", target_bir_lowering=False, debug=False,
                   num_devices=NCORES)

    # Inputs are host-swizzled to partition-major layout ([128, c, cols],
    # contiguous per partition) so every DMA moves 2-8 KB runs with ~128
    # descriptors: issue cost ~0.7us and full per-engine DMA rate (256-byte
    # runs measure ~6x slower per byte).
    xsw_d = nc.dram_tensor("xsw", [T // 512, 128, CT, 512], BF16,
                           kind="ExternalInput")
    # chunk 0's column halves as their own contiguous arrays: slicing
    # columns out of xsw would fragment the DMA into 1024 short descriptors
    # (~5us issue, ~1/3 rate); these stay 128-descriptor/4KB-run transfers
    x0a_d = nc.dram_tensor("x0a", [128, CT, 256], BF16, kind="ExternalInput")
    x0b_d = nc.dram_tensor("x0b", [128, CT, 256], BF16, kind="ExternalInput")
    wv_d = nc.dram_tensor("wv", [128, CT, HL], BF16, kind="ExternalInput")
    wqk_d = nc.dram_tensor("wqk", [128, CT, 2 * HL], BF16,
                           kind="ExternalInput")
    bq_d = nc.dram_tensor("bq", [HL], FP32, kind="ExternalInput")
    wp_d = nc.dram_tensor("wp", [128, NQO, C], BF16, kind="ExternalInput")
    out_d = nc.dram_tensor("out", [T, C], FP32, kind="ExternalOutput")

    with tile.TileContext(nc) as tc, ExitStack() as ctx:
        xt = ctx.enter_context(tc.tile_pool(name="xt", bufs=1))
        wq = ctx.enter_context(tc.tile_pool(name="wq", bufs=1))
        wp = ctx.enter_context(tc.tile_pool(name="wp", bufs=1))
        qk = ctx.enter_context(tc.tile_pool(name="qk", bufs=2 * NQO))
        vv = ctx.enter_context(tc.tile_pool(name="vv", bufs=(TT + 3) // 4))
        es = ctx.enter_context(tc.tile_pool(name="es", bufs=4))
        yt = ctx.enter_context(tc.tile_pool(name="yt", bufs=NQO))
        ob = ctx.enter_context(tc.tile_pool(name="ob", bufs=3))
        bc = ctx.enter_context(tc.tile_pool(name="bc", bufs=3))
        sc = ctx.enter_context(tc.tile_pool(name="sc", bufs=1))
        # PSUM budget (8 banks): qkv/V 2x[128,512]=2, scores/proj 2x[128,1024]=4,
        # attV accumulators 2x[65,512]=2. Separate tags so the second pair's
        # qkv matmuls can fill PE gaps while attention waits on softmax.
        pq = ctx.enter_context(tc.tile_pool(name="pq", bufs=2, space="PSUM"))
        ss = ctx.enter_context(tc.tile_pool(name="ss", bufs=2, space="PSUM"))
        py = ctx.enter_context(tc.tile_pool(name="py", bufs=2, space="PSUM"))

        # ---- inputs -> SBUF ----
        # Few BIG strided DMAs (each dma_start costs ~650ns of queue issue
        # time), ordered so the first V matmul's inputs land first: v-weight
        # slice, then x in 512-column t-chunks (chunk 0 is all the V t-tiles
        # 0-3 and the first QK chunk need), interleaved across two queues.
        # The DMA engines drain each queue's transfers in order, and both
        # queues share the same ~400 GB/s of engine bandwidth, so completion
        # time of any piece ~ tracks total bytes ahead of it. The critical
        # pieces (v-weights + x chunk 0) lead on two queues; x chunks 2-3,
        # the q/k weights and w_proj are issued from the DVE queue
        # mid-V-phase so they do not compete for bandwidth until the pieces
        # ahead of them have landed.
        wqall = wq.tile([128, CT, 3 * HL], BF16, tag="wq", name="wtile")
        xtall = xt.tile([128, CT, T], BF16, tag="xt", name="xtile")
        wpall = wp.tile([128, NQO, C], BF16, tag="wp", name="wptile")
        bq_sb = sc.tile([128, NQO], FP32, tag="bq")

        # The first matmul needs BOTH the v-weights and the x chunk-0 head:
        # lead with one on each queue so they transfer concurrently instead
        # of back-to-back. Everything else rides behind in consumption
        # order; in-order queue completion keeps the late pieces (wqk, wp,
        # bq — not needed until QK starts ~25us in) off the DMA engines
        # while the V phase races for the early x chunks.
        nc.sync.dma_start(out=xtall[:, :, 0:256], in_=x0a_d.ap())
        nc.scalar.dma_start(out=wqall[:, :, 2 * HL:3 * HL], in_=wv_d.ap())
        nc.sync.dma_start(out=xtall[:, :, 256:512], in_=x0b_d.ap())
        nc.sync.dma_start(out=xtall[:, :, 512:1024], in_=xsw_d.ap()[1])
        nc.scalar.dma_start(out=xtall[:, :, 1024:1536], in_=xsw_d.ap()[2])
        nc.sync.dma_start(out=xtall[:, :, 1536:2048], in_=xsw_d.ap()[3])
        nc.scalar.dma_start(out=wqall[:, :, 0:2 * HL], in_=wqk_d.ap())
        nc.scalar.dma_start(out=wpall, in_=wp_d.ap())
        nc.scalar.dma_start(out=bq_sb,
                            in_=bq_d.ap().rearrange("(j p) -> p j", p=128))
        wqs = [wqall[:, c] for c in range(CT)]
        xts = [xtall[:, c] for c in range(CT)]
        wps = [wpall[:, i] for i in range(NQO)]

        # ones source for V's denominator column (ACT rounds fp32->bf16)
        ones_sb = sc.tile([128, 4 * HPC], FP32, tag="ones")
        nc.gpsimd.memset(ones_sb, 1.0)
        # lower-triangular 0/1 constant: the causal mask becomes a cheap DVE
        # multiply, keeping gpsimd free so partition_broadcast dispatches the
        # moment its reciprocal lands (gpsimd's in-order queue otherwise
        # delays it behind exp-paced work, convoying the DVE normalize)
        tri = sc.tile([128, 128], BF16, tag="tri")
        nc.gpsimd.memset(tri, 1.0)
        nc.gpsimd.affine_select(
            out=tri, in_=tri, compare_op=mybir.AluOpType.is_ge,
            fill=0.0, base=0, pattern=[[1, 128]], channel_multiplier=-1,
        )
        vts = []
        for g in range((TT + 3) // 4):
            vt = vv.tile([128, 4, HPC, D + 1], BF16, tag="vv", name="vtile")
            nc.scalar.copy(
                vt[:, :, :, D],
                ones_sb.rearrange("p (a b) -> p a b", a=4),
            )
            vts.append(vt)

        qk_tiles = [qk.tile([128, T], BF16, tag="qk", name="qktile")
                    for _ in range(2 * NQO)]
        yts = [yt.tile([128, T], BF16, tag="yt", name="ytile")
               for _ in range(NQO)]
        # softmax denominators, all on partition 0 (one 512-slot per head
        # x q-chunk) so partition_broadcast reads them without a bounce DMA
        dstage = sc.tile([1, HPC * NCH * 512], FP32, tag="dstage")
        dstage_r = sc.tile([1, HPC * NCH * 512], FP32, tag="dstage_r")

        # ---- V for all heads (N=256 keeps the PE at full rate) ----
        for tt in range(TT):
            pv = pq.tile([128, 512], FP32, tag="pq", name="pv")
            for c in range(CT):
                nc.tensor.matmul(
                    pv[:, 0:HL],
                    xts[c][:, tt * 128:(tt + 1) * 128],
                    wqs[c][:, 2 * HL:3 * HL],
                    start=(c == 0), stop=(c == CT - 1),
                )
            nc.vector.tensor_copy(
                vts[tt // 4][:, tt % 4, :, 0:D],
                pv[:, 0:HL].rearrange("p (h d) -> p h d", h=HPC),
            )


        def emit_qk_pair(pair):
            for o in (pair, NQO + pair):
                col0 = o * 128 if o < NQO else HL + (o - NQO) * 128
                for tch in range(T // 512):
                    pt = pq.tile([128, 512], FP32, tag="pq", name="pqk")
                    for c in range(CT):
                        nc.tensor.matmul(
                            pt,
                            wqs[c][:, col0:col0 + 128],
                            xts[c][:, tch * 512:(tch + 1) * 512],
                            start=(c == 0), stop=(c == CT - 1),
                        )
                    dst = qk_tiles[o][:, tch * 512:(tch + 1) * 512]
                    if o < NQO:  # add q bias (per-partition)
                        nc.vector.tensor_scalar_add(dst, pt, bq_sb[:, o:o + 1])
                    else:
                        nc.vector.tensor_copy(dst, pt)

        def emit_cproj(tts, final=False):
            for tt in tts:
                po = ss.tile([128, 1024], FP32, tag="ss", name="po")
                for s in range(2):
                    for i in range(NQO):
                        nc.tensor.matmul(
                            po[:, s * 512:(s + 1) * 512],
                            yts[i][:, tt * 128:(tt + 1) * 128],
                            wps[i][:, s * 512:(s + 1) * 512],
                            start=(i == 0), stop=(i == NQO - 1),
                        )
                ot = ob.tile([128, C], FP32, tag="ob", name="otile")
                # split the PSUM->SBUF copy across ACT and DVE so the po
                # tile recycles at matmul pace. Out-DMAs stay on the sync
                # queue mid-kernel (a dma_start costs ~650ns of issue time
                # and would delay the exps queued on the scalar engine);
                # only the final batch spreads across both queues to drain
                # the tail faster.
                nc.scalar.copy(ot[:, 0:512], po[:, 0:512])
                nc.vector.tensor_copy(ot[:, 512:1024], po[:, 512:1024])
                if final:
                    nc.sync.dma_start(
                        out=out_d[tt * 128:(tt + 1) * 128, 0:512],
                        in_=ot[:, 0:512])
                    nc.scalar.dma_start(
                        out=out_d[tt * 128:(tt + 1) * 128, 512:1024],
                        in_=ot[:, 512:1024])
                else:
                    nc.sync.dma_start(
                        out=out_d[tt * 128:(tt + 1) * 128, :], in_=ot)

        def emit_attention_head(pair, h01):
            last_head = (pair == NQO - 1 and h01 == 1)
            hb = 64 * h01
            h = 2 * pair + h01          # local head index 0..3
            qt = qk_tiles[pair]
            kt_tile = qk_tiles[NQO + pair]

            for half in range(2):
                q0, q1 = half * HALF, (half + 1) * HALF
                py_map = {}
                for kt in range(q1 // 128):
                    qa = max(kt * 128, q0)
                    w = q1 - qa
                    qa0 = (qa // 512) * 512
                    pt = ss.tile([128, 1024], FP32, tag="ss", name="pst")
                    off = 0
                    for cw in _nsplit(w):
                        nc.tensor.matmul(
                            pt[:, off:off + cw],
                            kt_tile[hb:hb + 64, kt * 128:(kt + 1) * 128],
                            qt[hb:hb + 64, qa + off:qa + off + cw],
                            start=True, stop=True,
                        )
                        off += cw
                    es_t = es.tile([128, 1024], BF16, tag="es", name="estile")
                    nc.scalar.activation(
                        es_t[:, qa - qa0:qa - qa0 + w], pt[:, 0:w],
                        AF.Exp, scale=0.125,
                    )
                    if qa == kt * 128:
                        # causal mask: zero exp values where k > q in the
                        # diagonal block (bf16 2x DVE multiply by the
                        # triangular constant)
                        nc.vector.tensor_mul(
                            es_t[:, qa - qa0:qa - qa0 + 128],
                            es_t[:, qa - qa0:qa - qa0 + 128],
                            tri,
                        )
                    for cg in range(q0 // 512, q1 // 512):
                        if kt * 128 >= (cg + 1) * 512:
                            continue
                        if cg not in py_map:
                            py_map[cg] = py.tile([65, 512], FP32,
                                                 tag="py", name="pyt")
                        last_kt = min(q1 // 128, (cg + 1) * 4) - 1
                        # clip to causally-valid columns (q >= kt*128)
                        c0 = max(cg * 512, kt * 128)
                        nc.tensor.matmul(
                            py_map[cg][:, c0 - cg * 512:512],
                            vts[kt // 4][:, kt % 4, h, :],
                            es_t[:, c0 - qa0:(cg + 1) * 512 - qa0],
                            start=(kt == 0), stop=(kt == last_kt),
                        )
                        if kt == last_kt:
                            # stage unnormalized y + denominator row, release
                            # the PSUM slot; approx-recip on DVE, broadcast on
                            # the (otherwise empty) gpsimd queue, in-place
                            # multiply back on DVE
                            py_t = py_map[cg]
                            nc.vector.tensor_copy(
                                yts[pair][hb:hb + 64,
                                          cg * 512:(cg + 1) * 512],
                                py_t[0:64, :],
                            )
                            slot = (h * NCH + cg) * 512
                            nc.vector.tensor_copy(
                                dstage[0:1, slot:slot + 512],
                                py_t[64:65, :])
                            nc.vector.reciprocal_approx_fast(
                                out=dstage_r[0:1, slot:slot + 512],
                                in_=dstage[0:1, slot:slot + 512])
                            bc_t = bc.tile([128, 512], FP32, tag="bc",
                                           name="bct")
                            nc.gpsimd.partition_broadcast(
                                bc_t, dstage_r[0:1, slot:slot + 512])
                            dst = yts[pair][hb:hb + 64,
                                            cg * 512:(cg + 1) * 512]
                            nc.vector.tensor_mul(dst, dst,
                                                 bc_t[hb:hb + 64, :])
                if last_head and half == 0:
                    # first half of every head's y is final: emit c_proj for
                    # rows 0..T/2 here so its matmuls fill half-1's softmax
                    # waits and only half the c_proj remains in the tail
                    emit_cproj(range(TT // 2))

        for pair in range(NQO):
            emit_qk_pair(pair)
            for h01 in range(2):
                emit_attention_head(pair, h01)

        emit_cproj(range(TT // 2, TT), final=True)

    nc.compile()  # bacc lowering: register allocation, library/ACT table loads
    return nc


_NC_CACHE = {}


def _get_nc(T=T_FULL):
    if T not in _NC_CACHE:
        _NC_CACHE[T] = build_bass(T)
    return _NC_CACHE[T]


def _swz(a2d):
    """[c*128+p, cols] row-major -> [128, c, cols] contiguous per partition
    (the SBUF layout, so DMA runs are maximal)."""
    r, cols = a2d.shape
    return np.ascontiguousarray(
        a2d.reshape(r // 128, 128, cols).transpose(1, 0, 2))


def make_in_maps(x, w_attn, b_attn, w_proj, T=T_FULL):
    x = np.ascontiguousarray(np.asarray(x, np.float32))
    w_attn = np.asarray(w_attn, np.float32)
    b_attn = np.asarray(b_attn, np.float32)
    w_proj = np.asarray(w_proj, np.float32)
    # xsw[k, p, c, t] = x[b].T[c*128+p, 512k+t]
    xsws = [np.ascontiguousarray(
                x[b].T.astype(bfloat16)
                .reshape(C // 128, 128, T // 512, 512)
                .transpose(2, 1, 0, 3))
            for b in range(x.shape[0])]
    in_maps = []
    for core in range(NCORES):
        b, j = core // CPG, core % CPG
        r0 = j * HL
        wq_s = w_attn[r0:r0 + HL]
        wk_s = w_attn[C + r0:C + r0 + HL]
        wv_s = w_attn[2 * C + r0:2 * C + r0 + HL]
        in_maps.append({
            "xsw": xsws[b],
            "x0a": np.ascontiguousarray(xsws[b][0][:, :, 0:256]),
            "x0b": np.ascontiguousarray(xsws[b][0][:, :, 256:512]),
            "wv": _swz(np.ascontiguousarray(wv_s.T).astype(bfloat16)),
            "wqk": _swz(np.ascontiguousarray(
                np.concatenate([wq_s, wk_s], axis=0).T).astype(bfloat16)),
            "bq": np.ascontiguousarray(b_attn[r0:r0 + HL]),
            "wp": _swz(np.ascontiguousarray(
                w_proj[:, r0:r0 + HL].T).astype(bfloat16)),
        })
    return in_maps


def run_device(x, w_attn, b_attn, w_proj, b_proj, T=T_FULL, **spmd_kwargs):
    nc = _get_nc(T)
    in_maps = make_in_maps(x, w_attn, b_attn, w_proj, T)
    res = run_bass_kernel_spmd(nc, in_maps, core_ids=list(range(NCORES)),
                               **spmd_kwargs)
    outs = [r["out"] for r in res.results]
    b_eff = (np.asarray(b_proj, np.float32)
             + np.asarray(w_proj, np.float32) @ np.asarray(b_attn, np.float32)[2 * C:])
    full = np.stack(
        [sum(outs[b * CPG:(b + 1) * CPG][1:], outs[b * CPG]) + b_eff
         for b in range(B)]
    ).astype(np.float32)
    return full, res


def kernel(x, w_attn, b_attn, w_proj, b_proj):
    out, _ = run_device(x, w_attn, b_attn, w_proj, b_proj)
    return out
